# revision 96
# baseline (speedup 1.0000x reference)
"""Trainium2 Bass kernel for causal self-attention with RoPE (nn_CausalSelfAttention).

Problem (hardcoded): B=2, S=2048, D=1024, H=16 heads, head_dim=64, fp32,
causal mask, RoPE (rotate-half, base 10000), torch-Linear projections
q = x @ Wq.T, kv = x @ Wkv.T interleaved (k even, v odd output channels).

Sharding: 8 cores = 2 batches x 4 head-groups (4 heads each, as 2 row-packed
pairs). Everything per-core is local; no collectives.

Device-side layout choices:
  - All matmul operands are bf16 (PSUM accumulation stays fp32): same PE
    stream rate as f32r at wide tiles, but no 4x penalty below 256-wide
    moving dims, so diagonal score/AV chunks can be causally truncated.
  - All projection activations x are fed transposed (d_in on partitions),
    DRAM-laid-out so each seq-wave loads with ONE descriptor-cheap DMA
    (HWDGE costs ~625ns per dma_start regardless of size -- batch hard).
  - q,k are produced TRANSPOSED per head-pair: (128 partitions = 2 heads x 64
    dims, seq free) -- directly the scores lhsT/rhs layout.
  - Head dims are permuted on partitions ("paired d-order") so the RoPE
    rotate-half partner is always +16 mod 32 within a 32-partition quadrant,
    implementable with a single DVE stream_shuffle.
  - Scores are computed transposed S^T[k, q] per 128-k-chunk with 2 heads
    (contraction=64 each), truncated to the causally live [lo:] columns,
    into PER-HEAD 1-bank PSUM tiles (ring of 4) so each head's
    scores->exp->mask->AV chain releases PSUM independently.
  - softmax without max-subtraction (scores ~ N(0,1), |s|<~7 -- safe). Exp
    is SPLIT ACROSS ENGINES per chunk: one head exact-exp on ScalarE, the
    other on DVE via a Schraudolph bit-trick (i16 = rint(s*A + B), bitcast
    bf16 = exp with |eps| <= 3.1%, centered); heads alternate engines per
    k-chunk so each head's P is only ~50% approximate (softmax
    normalization cancels most of it; measured end-to-end rel ~7e-3, same
    as all-exact bf16). q-block 0 keeps both heads exact: its rows attend
    over few positions so per-weight noise does not average out there.
    Causal tri masks both run on DVE (bf16 4x) -- keeping them off GPSIMD
    matters because the Pool queue is busy with 1us rope multiplies.
  - AV: out^T[d, q] accumulated over k-chunks in PSUM; v carries an extra
    ones-column so row 64 accumulates sum(exp) for free.
  - Next-wave projection work is spliced between attention chunks through a
    filler queue so the in-order PE stream never idles while exps run.
    RoPE is spread over three engines: shuffle+cos-mul on DVE (PSUM reads),
    sin-mul on GPSIMD, final add on GPSIMD (SBUF-only operands).
  - Inputs are packed for the single-HWDGE-resource DMA model (~625ns per
    dma_start): q/k weights interleaved per-kc in consumption order in one
    `wqk` tensor, cos/sin/tri packed wave-0-first in one `tbl` tensor, 14
    input dma_starts total, ordered so the PE consumes each piece as it
    lands. The final output flush splits its two DMAs across the SP and
    Act queues so the transfers overlap the staging copies.
  - Normalization + final transpose on host (cheap numpy) from the returned
    (heads, 65, S) tensor.
"""

import numpy as np

B, S, D = 2, 2048, 1024
H, HD = 16, 64
NCORES = 8
ROPE_BASE = 10000.0
NKC = D // 128          # contraction chunks for projections (8)
NSC = S // 128          # seq chunks of 128 (16)
NW = 4                  # seq waves of 512

_CACHE = {}


# --------------------------------------------------------------------------
# host-side index maps
# --------------------------------------------------------------------------
def _dperm():
    """Row r (0..63) -> head-dim d, arranged so the rotate-half partner of the
    dim at row r sits at row (r//32)*32 + (r%32+16)%32 (same quadrant)."""
    p = np.empty(64, np.int64)
    for r in range(64):
        quad, i = divmod(r, 32)
        p[r] = 16 * quad + i if i < 16 else 32 + 16 * quad + (i - 16)
    return p


def _rope_tables():
    inv = 1.0 / (ROPE_BASE ** (np.arange(0, HD, 2, dtype=np.float64) / HD))  # (32,)
    t = np.arange(S, dtype=np.float64)
    fr = t[:, None] * inv[None, :]                    # (S, 32)
    return np.cos(fr), np.sin(fr)                     # float64 (S, 32)


# --------------------------------------------------------------------------
# device kernel builder (same NEFF for all 8 cores)
# --------------------------------------------------------------------------
def _build(reps=1, timing=False):
    key = ("nc", reps, timing)
    if key in _CACHE:
        return _CACHE[key]
    import concourse.tile as tile
    from concourse import bacc, mybir

    f32 = mybir.dt.float32
    bf16 = mybir.dt.bfloat16
    i16 = mybir.dt.int16
    EXP = mybir.ActivationFunctionType.Exp
    MUL = mybir.AluOpType.mult
    ADD = mybir.AluOpType.add
    # Schraudolph exp-on-DVE: i16 = rint(f32(s*A) + B); bitcast int16->bf16
    # computes exp(0.125*s)*(1+eps), |eps|<=3.1% (centered by C_MAGIC), which
    # softmax normalization largely cancels (measured end-to-end rel ~5e-3).
    A_SCHR = float(0.125 * 128.0 / np.log(2.0))
    B_SCHR = float(127.0 * 128.0 - 5.65)

    nc = bacc.Bacc("TRN2", target_bir_lowering=False, debug=False)
    kin = "Internal" if timing else "ExternalInput"
    kout = "Internal" if timing else "ExternalOutput"
    xT = nc.dram_tensor("xT", [128, NW, NKC, 512], bf16, kind=kin).ap()
    # wqk: q/k weights packed in consumption order: [pair, kc, {q,k}, 128]
    wqk = nc.dram_tensor("wqk", [128, 2, NKC, 2, 128], bf16, kind=kin).ap()
    wv = nc.dram_tensor("wv", [128, NKC, 256], bf16, kind=kin).ap()
    # tbl: [cos_w0 512 | sin_w0 512 | tri 128 | cos_w123 1536 | sin_w123 1536]
    tbl = nc.dram_tensor("tbl", [128, 4224], bf16, kind=kin).ap()
    o = nc.dram_tensor("o", [4, 65, S], bf16, kind=kout).ap()
    if timing:
        dummy_in = nc.dram_tensor("dummy_in", [1, 64], f32, kind="ExternalInput").ap()
        dummy_out = nc.dram_tensor("dummy_out", [1, 64], f32, kind="ExternalOutput").ap()

    shuf_mask = [(i + 16) % 32 for i in range(32)]

    with tile.TileContext(nc) as tc:
        with (
            tc.tile_pool(name="cst", bufs=1) as cst,
            tc.tile_pool(name="rope", bufs=10) as rope,
            tc.tile_pool(name="ptp", bufs=22) as ptp,
            tc.tile_pool(name="ost", bufs=6) as ost,
            tc.tile_pool(name="pps", bufs=2, space="PSUM") as pps,
            tc.tile_pool(name="scp", bufs=2, space="PSUM") as scp,
            tc.tile_pool(name="ops", bufs=1, space="PSUM") as ops,
        ):
            xT_sbs = [cst.tile([128, NKC, 512], bf16, tag=f"xT{i}",
                               name=f"xT_sb{i}") for i in range(4)]
            wqk_sb = cst.tile([128, 2, NKC, 2, 128], bf16, tag="wqk")
            wv_sb = cst.tile([128, NKC, 256], bf16, tag="wv")
            tbl_sb = cst.tile([128, 4224], bf16, tag="tbl")
            qT_sb = cst.tile([128, 2, S], bf16, tag="qT")
            kT_sb = cst.tile([128, 2, S], bf16, tag="kT")
            vx_sb = cst.tile([128, NSC, 4, 65], bf16, tag="vx")
            warm = cst.tile([128, 16], bf16, tag="warm")

            tri_sb = tbl_sb[:, 1024:1152]

            def cos_ap(sb):
                return (tbl_sb[:, 0:512] if sb == 0
                        else tbl_sb[:, 1152 + (sb - 1) * 512:1152 + sb * 512])

            def sin_ap(sb):
                return (tbl_sb[:, 512:1024] if sb == 0
                        else tbl_sb[:, 2688 + (sb - 1) * 512:2688 + sb * 512])

            # ------------------------------------------------------------
            # projection emission units (q/k with RoPE, v with copy)
            # ------------------------------------------------------------
            def qk_units(dst, qk, t, sb, rp, which, add_dve=False):
                """Units (pe_ns, closure): 4x 2-kc matmul chunks + RoPE tail."""
                ps = pps.tile([128, 512], f32, tag="proj",
                              name=f"ps_{which}_{rp}_{t}_{sb}")

                def mm(kc):
                    nc.tensor.matmul(
                        ps[:],
                        wqk_sb[:, t, kc, qk, :],
                        xT_sbs[sb][:, kc, :],
                        start=(kc == 0), stop=(kc == NKC - 1))

                def rope_tail():
                    # m1 right after the shuffle: both readers of ps done
                    # ASAP, releasing the PSUM buffer for the next group.
                    sl = slice(sb * 512, (sb + 1) * 512)
                    shf = rope.tile([128, 512], f32, tag="shf")
                    nc.vector.stream_shuffle(shf[:], ps[:], shuf_mask)
                    m1 = rope.tile([128, 512], f32, tag="m1")
                    nc.vector.tensor_tensor(m1[:], ps[:], cos_ap(sb), MUL)
                    m2 = rope.tile([128, 512], f32, tag="m2")
                    nc.gpsimd.tensor_tensor(m2[:], shf[:], sin_ap(sb), MUL)
                    # prologue tails: the add on DVE (533 vs 1219ns) -- qT/kT
                    # wave-0 readiness gates the first attention block.
                    eng = nc.vector if add_dve else nc.gpsimd
                    eng.tensor_add(dst[:, t, sl], m1[:], m2[:])

                return [(213, lambda kc=kc: mm(kc))
                        for kc in range(NKC)] + [(0, rope_tail)]

            def v_units(sc, rp):
                """Units (pe_ns, closure): 2x 4-kc matmul chunks + copy tail."""
                psv = pps.tile([128, 512], f32, tag="proj",
                               name=f"psv_{rp}_{sc}")

                def mm(kc0):
                    for kc in range(kc0, kc0 + 4):
                        nc.tensor.matmul(
                            psv[:, 0:256],
                            xT_sbs[sc // 4][:, kc, (sc % 4) * 128:(sc % 4 + 1) * 128],
                            wv_sb[:, kc, :],
                            start=(kc == 0), stop=(kc == NKC - 1))

                def copy_tail():
                    nc.vector.tensor_copy(
                        vx_sb[:, sc, :, 0:64],
                        psv[:, 0:256].rearrange("p (h d) -> p h d", h=4))

                return [(428, lambda kc0=kc0: mm(kc0)) for kc0 in (0, 4)] \
                    + [(0, copy_tail)]

            # Filler queue: next-wave projection emission is spliced between
            # attention chunks so the in-order PE stream interleaves it with
            # attention instead of idling while ScalarE runs exp.
            fill_q = []

            def fill(budget_ns, max_pops=6):
                # Pop units until ~budget_ns of PE work has been spliced in
                # (RoPE/copy tails carry no PE work and ride along free).
                popped = 0
                pops = 0
                while fill_q and popped < budget_ns and pops < max_pops:
                    pe_ns, fn = fill_q.pop(0)
                    fn()
                    popped += pe_ns
                    pops += 1

            def drain_fill():
                while fill_q:
                    fill_q.pop(0)[1]()

            def run_now(units):
                for _, fn in units:
                    fn()

            def queue(*unit_lists):
                for us in unit_lists:
                    fill_q.extend(us)

            # ------------------------------------------------------------
            # attention
            # ------------------------------------------------------------
            def attn_qb(pair, qb, rp):
                qlo = qb * 512
                o_ps = [ops.tile([65, 512], f32, tag=f"o{h}",
                                 name=f"o_ps{rp}_{pair}_{qb}_{h}")
                        for h in range(2)]
                nchunks = 4 * qb + 4

                def emit_sc(c):
                    s = c - 4 * qb        # >=0 on diagonal chunks
                    lo = 0 if s < 0 else 128 * s
                    sc_h = []
                    for h in range(2):
                        t = scp.tile([128, 512], f32, tag="sc", bufs=4,
                                     name=f"sc_{rp}_{pair}_{qb}_{c}_{h}")
                        nc.tensor.matmul(
                            t[:, lo:],
                            kT_sb[h * 64:(h + 1) * 64, pair,
                                  c * 128:(c + 1) * 128],
                            qT_sb[h * 64:(h + 1) * 64, pair,
                                  qlo + lo:qlo + 512],
                            start=True, stop=True)
                        sc_h.append(t)
                    return sc_h

                def emit_post(c, sc_h):
                    # Per-head engine split: one head's exp exact on ScalarE,
                    # the other Schraudolph on DVE. Each head's chain (scores
                    # -> exp -> tri -> AV) stays on independent engines with
                    # its own 1-bank PSUM ring slot, so neither engine's
                    # latency can stall the other's and the PE never waits on
                    # a shared score-tile release. The heads ALTERNATE engines
                    # per k-chunk so every head's P is only ~50% approximate
                    # (full-schr heads would cost 3x the end-to-end error).
                    s = c - 4 * qb
                    lo = 0 if s < 0 else 128 * s
                    hs = c % 2            # head served by ScalarE this chunk
                    pt = ptp.tile([128, 2, 512], bf16, tag="pt")
                    nc.scalar.activation(
                        pt[:, hs, lo:], sc_h[hs][:, lo:], EXP, scale=0.125)
                    if qb == 0:
                        # Early rows attend over few positions, so per-weight
                        # exp error does not average out -- q-block 0 keeps
                        # exact exp for BOTH heads (it is filler-rich, so
                        # ScalarE serialization is hidden).
                        nc.scalar.activation(
                            pt[:, 1 - hs, lo:], sc_h[1 - hs][:, lo:], EXP,
                            scale=0.125)
                    else:
                        nc.vector.tensor_scalar(
                            pt[:, 1 - hs, lo:].bitcast(i16),
                            sc_h[1 - hs][:, lo:],
                            A_SCHR, B_SCHR, MUL, ADD)
                    if s >= 0:
                        # Both tri masks on DVE (cheap at bf16 4x mode); the
                        # DVE-exp head's tri first so DVE never head-of-line
                        # blocks waiting for the ScalarE head's exp.
                        nc.vector.tensor_tensor(
                            pt[:, 1 - hs, lo:lo + 128],
                            pt[:, 1 - hs, lo:lo + 128],
                            tri_sb, MUL)
                        nc.vector.tensor_tensor(
                            pt[:, hs, lo:lo + 128], pt[:, hs, lo:lo + 128],
                            tri_sb, MUL)
                    return pt, lo

                def emit_av(c, pt, lo):
                    for h in range(2):
                        nc.tensor.matmul(
                            o_ps[h][:, lo:512],
                            vx_sb[:, c, 2 * pair + h, :],
                            pt[:, h, lo:512],
                            start=(c == 0), stop=(c == nchunks - 1))

                def flush(final=False, on_scal=False):
                    # One SBUF staging tile + ONE output DMA per q-block
                    # (each dma_start costs a full HWDGE slot). The very
                    # last flush splits copies across DVE and ScalarE AND
                    # splits the DMA by head onto two queues (SP + Act) so
                    # the h0 transfer overlaps the h1 copy. Tail-block
                    # flushes go on ScalarE (DVE carries the tail exp load).
                    o_sb = ost.tile([65, 2, 512], bf16, tag="ost")
                    if final:
                        # one combined DMA: a per-head dma pair would pay
                        # 2x the serialized 625ns HWDGE slot to save only
                        # ~185ns of transfer
                        nc.vector.tensor_copy(o_sb[:, 0, :], o_ps[0][:])
                        nc.scalar.copy(o_sb[:, 1, :], o_ps[1][:])
                        nc.sync.dma_start(
                            o[2 * pair:2 * pair + 2, :, qlo:qlo + 512]
                            .rearrange("h p q -> p h q"),
                            o_sb[:])
                    else:
                        nc.vector.tensor_copy(o_sb[:, 0, :], o_ps[0][:])
                        nc.scalar.copy(o_sb[:, 1, :], o_ps[1][:])
                        nc.sync.dma_start(
                            o[2 * pair:2 * pair + 2, :, qlo:qlo + 512]
                            .rearrange("h p q -> p h q"),
                            o_sb[:])

                return emit_sc, emit_post, emit_av, flush, nchunks

            def attn_qb_run(pair, qb, rp, rate=1, final=False,
                            flush_scal=False, prefill=0):
                # Software-pipelined emission: scores(c+1) are emitted BEFORE
                # AV(c) so the in-order PE stream never blocks on exp(c) with
                # the next chunk's scores still unissued; filler units keep PE
                # fed while ScalarE works. `rate` = filler units per chunk
                # (int, or per-chunk list) -- ~1 matches the exp-latency gap.
                # `prefill` splices filler between the first scores and AV(0)
                # (block 0 uses this to emit the v projections its own AV(0)
                # depends on -- they must precede AV(0) in the in-order PE
                # stream).
                emit_sc, emit_post, emit_av, flush, n = attn_qb(pair, qb, rp)
                sc_h = emit_sc(0)
                for c in range(n):
                    pt, lo = emit_post(c, sc_h)
                    if c + 1 < n:
                        sc_h = emit_sc(c + 1)
                    if c == 0 and prefill:
                        fill(prefill, max_pops=99)
                    # a small filler slice BEFORE av(c) buys exp(c) ~200ns of
                    # extra slack in the in-order PE stream (the remaining
                    # budget stays after so projection supply keeps pace)
                    fill(220, max_pops=1)
                    emit_av(c, pt, lo)
                    fill(rate[c] if isinstance(rate, (list, tuple)) else rate)
                flush(final=final, on_scal=flush_scal)

            if timing:
                dpool = cst.tile([1, 64], f32, tag="dumm", name="dumm")
                nc.sync.dma_start(dpool[:], dummy_in)
                nc.sync.dma_start(dummy_out, dpool[:])
            for rp in range(reps):
                # PE p-state warm-up: a tiny matmul on zeroed SBUF so the
                # engine's ramp window elapses during the input DMA.
                wm_ps = pps.tile([128, 512], f32, tag="proj",
                                 name=f"warm_ps{rp}")
                nc.gpsimd.memset(warm[:], 0.0)
                for _ in range(8):
                    nc.tensor.matmul(wm_ps[0:16, 0:16], warm[:], warm[:],
                                     start=True, stop=True)
                nc.gpsimd.memset(vx_sb[:, :, :, 64:65], 1.0)

                # Batched input DMA, consumption order. Each dma_start costs
                # ~625ns on the single HWDGE resource and all transfers
                # serialize on the DMA engines, so the prologue is packed
                # into few transfers laid out in exact consumption order
                # (wqk interleaves q/k per kc-pair; tbl packs wave-0 cos/sin
                # + tri ahead of the later waves).
                nc.sync.dma_start(wqk_sb[:, 0, 0:2], wqk[:, 0, 0:2])
                nc.sync.dma_start(xT_sbs[0][:, 0:2], xT[:, 0, 0:2])
                nc.sync.dma_start(wqk_sb[:, 0, 2:8], wqk[:, 0, 2:8])
                nc.sync.dma_start(xT_sbs[0][:, 2:4], xT[:, 0, 2:4])
                nc.sync.dma_start(xT_sbs[0][:, 4:6], xT[:, 0, 4:6])
                nc.sync.dma_start(xT_sbs[0][:, 6:8], xT[:, 0, 6:8])
                nc.sync.dma_start(tbl_sb[:, 0:1152], tbl[:, 0:1152])
                nc.sync.dma_start(wv_sb[:], wv)
                nc.sync.dma_start(wqk_sb[:, 1], wqk[:, 1])
                nc.sync.dma_start(xT_sbs[1][:], xT[:, 1])
                nc.sync.dma_start(tbl_sb[:, 1152:], tbl[:, 1152:])
                nc.sync.dma_start(xT_sbs[2][:], xT[:, 2])
                nc.sync.dma_start(xT_sbs[3][:], xT[:, 3])

                # Wave 0: pair-0 q/k matmuls interleaved per kc-pair (so PE
                # consumes each x0/w piece as it lands), then pair-1 q and all
                # v directly. Later projections are spliced into the attention
                # chunk stream at their LATEST legal position so the filler
                # supply reaches the attention tail.
                uq = qk_units(qT_sb, 0, 0, 0, rp, "q")
                uk = qk_units(kT_sb, 1, 0, 0, rp, "k")
                for i in range(0, 8, 2):
                    uq[i][1]()
                    uq[i + 1][1]()
                    uk[i][1]()
                    uk[i + 1][1]()
                uq[8][1]()
                uk[8][1]()
                for sc in range(0, 4):
                    run_now(v_units(sc, rp))
                run_now(qk_units(qT_sb, 0, 1, 0, rp, "q"))
                queue(qk_units(kT_sb, 1, 1, 0, rp, "k"),
                      qk_units(qT_sb, 0, 0, 1, rp, "q"))
                attn_qb_run(0, 0, rp, rate=900)
                drain_fill()
                queue(qk_units(kT_sb, 1, 0, 1, rp, "k"))
                attn_qb_run(1, 0, rp, rate=900)
                drain_fill()
                # (0,1): v1-sc4 first (needed at chunk 4), then next block's q
                # (so its RoPE lands mid-block, clear of boundary congestion)
                queue(v_units(4, rp),
                      qk_units(qT_sb, 0, 1, 1, rp, "q"),
                      v_units(5, rp), v_units(6, rp), v_units(7, rp),
                      qk_units(kT_sb, 1, 1, 1, rp, "k"))
                attn_qb_run(0, 1, rp, rate=[650] * 8 + [380] * 8)
                drain_fill()
                queue(qk_units(qT_sb, 0, 0, 2, rp, "q"),
                      qk_units(kT_sb, 1, 0, 2, rp, "k"))
                attn_qb_run(1, 1, rp, rate=450)
                drain_fill()
                # (0,2): v2 paced to land just before its chunks 8..11
                queue(v_units(8, rp),
                      qk_units(qT_sb, 0, 1, 2, rp, "q"),
                      v_units(9, rp), v_units(10, rp), v_units(11, rp))
                attn_qb_run(0, 2, rp, rate=[650] * 8 + [260] * 16)
                drain_fill()
                queue(qk_units(kT_sb, 1, 1, 2, rp, "k"),
                      qk_units(qT_sb, 0, 0, 3, rp, "q"),
                      qk_units(kT_sb, 1, 0, 3, rp, "k"))
                attn_qb_run(1, 2, rp, rate=[650] * 8 + [260] * 16)
                drain_fill()
                # (0,3): v3 paced to land just before its chunks 12..15
                queue(v_units(12, rp),
                      qk_units(qT_sb, 0, 1, 3, rp, "q"),
                      v_units(13, rp), v_units(14, rp), v_units(15, rp))
                attn_qb_run(0, 3, rp, rate=[650] * 6 + [240] * 10,
                            flush_scal=True)
                drain_fill()
                queue(qk_units(kT_sb, 1, 1, 3, rp, "k"))
                attn_qb_run(1, 3, rp, rate=240, final=True)

    nc.compile()
    _CACHE[key] = nc
    return nc


# --------------------------------------------------------------------------
# host-side sharding / unsharding
# --------------------------------------------------------------------------
def _make_in_maps(x, Wq, Wkv):
    import ml_dtypes
    BF = ml_dtypes.bfloat16

    x = np.asarray(x, np.float32)
    Wq = np.asarray(Wq, np.float32)
    Wkv = np.asarray(Wkv, np.float32)

    dp = _dperm()
    cos32, sin32 = _rope_tables()
    sign = np.where((np.arange(128) % 32) < 16, -1.0, 1.0)
    rows64 = np.concatenate([dp, dp])                       # 128 rows, 2 heads
    cosT = cos32[:, rows64 % 32].T.astype(BF)               # (128, S)
    sinT = (sin32[:, rows64 % 32].T * sign[:, None]).astype(BF)
    tri = (np.arange(128)[:, None] <= np.arange(128)[None, :]).astype(BF)
    # tbl pack: [cos_w0 | sin_w0 | tri | cos_w123 | sin_w123]
    tblc = np.ascontiguousarray(np.concatenate(
        [cosT[:, 0:512], sinT[:, 0:512], tri,
         cosT[:, 512:], sinT[:, 512:]], axis=1))            # (128, 4224)

    # (128, NW, NKC, 512): partition-major, wave-contiguous per partition
    xT_b = [np.ascontiguousarray(
        x[b].T.reshape(NKC, 128, NW, 512).transpose(1, 2, 0, 3)).astype(BF)
        for b in range(B)]

    in_maps = []
    for c in range(NCORES):
        b, g = divmod(c, 4)
        heads = [4 * g + hh for hh in range(4)]
        qrows = np.concatenate([h * 64 + dp for h in heads])
        krows = np.concatenate([h * 128 + 2 * dp for h in heads])
        vrows = np.concatenate([h * 128 + 2 * np.arange(64) + 1 for h in heads])
        wq_c = np.ascontiguousarray(
            Wq[qrows, :].T.reshape(NKC, 128, 256).transpose(1, 0, 2)).astype(BF)
        wk_c = np.ascontiguousarray(
            Wkv[krows, :].T.reshape(NKC, 128, 256).transpose(1, 0, 2)).astype(BF)
        wv_c = np.ascontiguousarray(
            Wkv[vrows, :].T.reshape(NKC, 128, 256).transpose(1, 0, 2)).astype(BF)
        # wqk: (128, pair, kc, {q,k}, 128) -- consumption order
        wqk_c = np.empty((128, 2, NKC, 2, 128), BF)
        for t in range(2):
            wqk_c[:, t, :, 0, :] = wq_c[:, :, t * 128:(t + 1) * 128]
            wqk_c[:, t, :, 1, :] = wk_c[:, :, t * 128:(t + 1) * 128]
        in_maps.append({
            "xT": xT_b[b], "wqk": np.ascontiguousarray(wqk_c),
            "wv": wv_c, "tbl": tblc,
        })
    return in_maps


def _assemble(results):
    out = np.empty((B, S, D), np.float32)
    for c in range(NCORES):
        b, g = divmod(c, 4)
        oc = np.asarray(results[c]["o"], np.float32)  # (4, 65, S)
        att = oc[:, :64, :] / oc[:, 64:65, :]         # (4, 64, S)
        for hh in range(4):
            head = 4 * g + hh
            out[b, :, head * 64:(head + 1) * 64] = att[hh].T
    return out


def kernel(x, Wq, Wkv, mask=None):
    from concourse.bass_utils import run_bass_kernel_spmd

    nc = _build()
    in_maps = _make_in_maps(x, Wq, Wkv)
    res = run_bass_kernel_spmd(nc, in_maps, core_ids=list(range(NCORES)))
    return _assemble(res.results)

